# revision 33
# baseline (speedup 1.0000x reference)
"""AMPNet GNN classifier on 8 Trainium2 NeuronCores (Bass/Tile).

Design:
  - Host: renumber nodes (degree-balanced 128-node blocks), sort edges by dst
    block, shard dst-blocks across 8 cores. Per-edge attention runs edge-major
    on DVE/ACT; projections / aggregation / output head run on TensorE.
  - Per conv layer: PE projections build q/kv tables in HBM (node-major rows),
    gpsimd dma_gather pulls per-edge q (by dst) and kv (by src) tiles,
    DVE computes per-edge MHA (products + grouped reduces + softmax),
    PE aggregates messages into per-block node sums via host-built one-hot
    indicator matmuls (output feature-major), then Wo + deg*bias + ELU.
  - AllGather (bf16) of per-core node features between the two convs.
  - Head: logits + log_softmax computed feature-major with K=1/K=7 matmul
    broadcast tricks; host reassembles/permutes the final [N, 7] output.
"""

import numpy as np
import ml_dtypes

import concourse.bacc as bacc
import concourse.bass as bass
import concourse.mybir as mybir
import concourse.tile as tile
from concourse.bass_utils import run_bass_kernel_spmd

N = 20000
E_TOT = 320000
F = 8
D = 16
H = 4
DH = 4
NOUT = 7
FDIM = 128  # F*D

NC = 8
NP = 20480          # padded node count (160 blocks of 128)
NB = 160            # total 128-node blocks
BPC = NB // NC      # blocks per core
NPC = NP // NC      # nodes per core
CH = 16             # edge tiles per gather chunk

BF16 = mybir.dt.bfloat16
F32 = mybir.dt.float32
I16 = mybir.dt.int16

AF = mybir.ActivationFunctionType
ALU = mybir.AluOpType


def _bd_qkv_weights(wqkv, bqkv):
    """Block-diagonal projection weights [128, 384] + bias row [1, 384].

    Output cols: 0:128   q-table row, col f*16 + (h*4+d)   (scaled by 1/sqrt(DH))
                 128:256 k part of kv row, col g*16 + (h*4+d)
                 256:384 v part of kv row, col h*32 + d*8 + g
    Input rows: token-major x row, f*16 + di.
    """
    wq, wk, wv = wqkv[0:16], wqkv[16:32], wqkv[32:48]
    bq, bk, bv = bqkv[0:16], bqkv[16:32], bqkv[32:48]
    sc = 1.0 / np.sqrt(DH)
    W = np.zeros((128, 384), np.float32)
    brow = np.zeros((1, 384), np.float32)
    # q row layout (f, d, h): col f*16 + d*4 + h
    # k row layout (g, d, h): col 128 + g*16 + d*4 + h
    # v row layout (d, g, h): col 256 + d*32 + g*4 + h
    for f in range(F):
        for h in range(H):
            for d in range(DH):
                W[f * 16:(f + 1) * 16, f * 16 + d * 4 + h] = wq[h * 4 + d] * sc
                W[f * 16:(f + 1) * 16, 128 + f * 16 + d * 4 + h] = wk[h * 4 + d]
                W[f * 16:(f + 1) * 16, 256 + d * 32 + f * 4 + h] = wv[h * 4 + d]
                brow[0, f * 16 + d * 4 + h] = bq[h * 4 + d] * sc
                brow[0, 128 + f * 16 + d * 4 + h] = bk[h * 4 + d]
                brow[0, 256 + d * 32 + f * 4 + h] = bv[h * 4 + d]
    return W, brow


def _bd_wo(wo):
    """Wo block-diag [128, 128]: in row f*16 + d*4 + h -> out col f*16+c."""
    W = np.zeros((128, 128), np.float32)
    for f in range(F):
        for h in range(H):
            for d in range(DH):
                W[f * 16 + d * 4 + h, f * 16:(f + 1) * 16] = wo[:, h * 4 + d]
    return W


def _host_forward(x, src, dst, p):
    """Exact reference forward in numpy (chunked). Returns max scores of both
    convs and max |logit| (for exp-shift constants), plus the logits."""
    def conv(hmat, wqkv, bqkv, wo, bo):
        wq, wk, wv = wqkv[0:16], wqkv[16:32], wqkv[32:48]
        bq, bk, bv = bqkv[0:16], bqkv[16:32], bqkv[32:48]
        h3 = hmat.reshape(N, F, D)
        q = (h3 @ wq.T + bq).astype(np.float32)
        k = (h3 @ wk.T + bk).astype(np.float32)
        v = (h3 @ wv.T + bv).astype(np.float32)
        agg = np.zeros((N, FDIM), np.float32)
        smax = -1e30
        order = np.argsort(dst, kind="stable")
        s_s, d_s = src[order], dst[order]
        msg = np.empty((E_TOT, FDIM), np.float32)
        CHK = 65536
        for lo in range(0, E_TOT, CHK):
            hi = min(lo + CHK, E_TOT)
            qd = q[d_s[lo:hi]].reshape(-1, F, H, DH)
            ks = k[s_s[lo:hi]].reshape(-1, F, H, DH)
            vs = v[s_s[lo:hi]].reshape(-1, F, H, DH)
            s = np.einsum("efhd,eghd->ehfg", qd, ks) * (1.0 / np.sqrt(DH))
            smax = max(smax, float(s.max()))
            s -= s.max(axis=-1, keepdims=True)
            es = np.exp(s)
            a = es / es.sum(axis=-1, keepdims=True)
            o = np.einsum("ehfg,eghd->efhd", a, vs).reshape(-1, FDIM)
            msg[lo:hi] = (o.reshape(-1, F, D) @ wo.T + bo).reshape(-1, FDIM)
        # segment sum over sorted dst
        uniq, starts = np.unique(d_s, return_index=True)
        sums = np.add.reduceat(msg, starts, axis=0)
        agg[uniq] = sums
        return agg, smax

    h1, smax1 = conv(x, p["w1qkv"], p["b1qkv"], p["w1o"], p["b1o"])
    h1 = np.where(h1 > 0, h1, np.expm1(np.minimum(h1, 0)))
    h2, smax2 = conv(h1, p["w2qkv"], p["b2qkv"], p["w2o"], p["b2o"])
    h2 = np.where(h2 > 0, h2, np.expm1(np.minimum(h2, 0)))
    logits = h2 @ p["out_w"].T + p["out_b"]
    return smax1, smax2, float(np.abs(logits).max()), logits, h1, h2


def _host_prep(inputs):
    x = np.asarray(inputs["x"], np.float32)
    ei = np.asarray(inputs["edge_index"], np.int64)
    src, dst = ei[0], ei[1]
    p = {
        "w1qkv": np.asarray(inputs["conv1_wqkv"], np.float32),
        "b1qkv": np.asarray(inputs["conv1_bqkv"], np.float32),
        "w1o": np.asarray(inputs["conv1_wo"], np.float32),
        "b1o": np.asarray(inputs["conv1_bo"], np.float32),
        "w2qkv": np.asarray(inputs["conv2_wqkv"], np.float32),
        "b2qkv": np.asarray(inputs["conv2_bqkv"], np.float32),
        "w2o": np.asarray(inputs["conv2_wo"], np.float32),
        "b2o": np.asarray(inputs["conv2_bo"], np.float32),
        "out_w": np.asarray(inputs["out_w"], np.float32),
        "out_b": np.asarray(inputs["out_b"], np.float32),
    }

    smax1, smax2, lmax, _logits_h, h1_h, h2_h = _host_forward(x, src, dst, p)
    c1 = max(0.0, smax1 - 10.0)
    c2 = max(0.0, smax2 - 10.0)
    c3 = max(0.0, lmax - 10.0)

    # ---- node renumbering: degree-balanced blocks (snake over sorted deg) ---
    deg = np.bincount(dst, minlength=N)
    order = np.argsort(-deg, kind="stable")
    rounds = N // NB  # 125
    pos = np.arange(N) % NB
    rnd = np.arange(N) // NB
    blk_of_rank = np.where(rnd % 2 == 0, pos, NB - 1 - pos)
    new_id_of_rank = blk_of_rank * 128 + rnd
    new_of_old = np.empty(N, np.int64)
    new_of_old[order] = new_id_of_rank

    s_new = new_of_old[src]
    d_new = new_of_old[dst]

    blk = d_new >> 7
    order_e = np.argsort(blk, kind="stable")
    s_s = s_new[order_e]
    d_s = d_new[order_e]
    blk_s = blk[order_e]
    cnt = np.bincount(blk_s, minlength=NB)
    T_BLK = int(np.ceil(cnt.max() / 128))
    EPB = T_BLK * 128
    NT0 = BPC * T_BLK
    NT = ((NT0 + CH - 1) // CH) * CH
    ES = NT * 128

    starts = np.zeros(NB, np.int64)
    starts[1:] = np.cumsum(cnt)[:-1]
    within = np.arange(E_TOT) - starts[blk_s]
    core_e = blk_s // BPC
    lb_e = blk_s % BPC
    slot = lb_e * EPB + within

    src_slots = np.zeros((NC, ES), np.int32)
    dst_slots = np.zeros((NC, ES), np.int32)
    # default dst for padding: base node of the slot's block (or last block)
    slot_blk = np.minimum(np.arange(ES) >> 7 >> 0, 0)  # placeholder
    for c in range(NC):
        lb_all = np.minimum(np.arange(ES) // EPB, BPC - 1)
        dst_slots[c] = (c * BPC + lb_all) * 128
    src_slots[core_e, slot] = s_s
    dst_slots[core_e, slot] = d_s

    ind = np.zeros((NC, 128, NT * 128), ml_dtypes.bfloat16)
    pp = slot & 127
    dloc = d_s - blk_s * 128
    tcol = (slot >> 7) * 128 + dloc
    ind[core_e, pp, tcol] = 1.0
    # transposed indicator for q-expansion: [node-in-block, edge-slot]
    ind2 = np.zeros((NC, 128, NT * 128), ml_dtypes.bfloat16)
    ind2[core_e, dloc, slot] = 1.0

    def wrap_idx(a):
        return np.ascontiguousarray(
            np.tile(a.reshape(-1, 16).T.astype(np.int16), (8, 1)))

    deg_new = np.bincount(d_new, minlength=NP).astype(np.float32)

    xp = np.zeros((NP, FDIM), np.float32)
    xp[new_of_old] = x
    xT = np.ascontiguousarray(xp.T).astype(ml_dtypes.bfloat16)

    W1, brow1 = _bd_qkv_weights(p["w1qkv"], p["b1qkv"])
    W2, brow2 = _bd_qkv_weights(p["w2qkv"], p["b2qkv"])
    Wo1 = _bd_wo(p["w1o"])
    Wo2 = _bd_wo(p["w2o"])
    bo_col1 = np.tile(p["b1o"], F)[None, :]  # [1,128] col f*16+c
    bo_col2 = np.tile(p["b2o"], F)[None, :]

    bf = ml_dtypes.bfloat16
    shared = {
        "xT": xT,
        "W1": W1.astype(bf), "W2": W2.astype(bf),
        "brow1": brow1.astype(bf), "brow2": brow2.astype(bf),
        "Wo1": Wo1.astype(bf), "Wo2": Wo2.astype(bf),
        "bocol1": bo_col1.astype(bf), "bocol2": bo_col2.astype(bf),
        "WoutT": np.ascontiguousarray(p["out_w"].T).astype(ml_dtypes.bfloat16),
        "outbc": (p["out_b"] - c3)[None, :].astype(np.float32),
        "nones7": np.full((1, NOUT), -1.0, np.float32),
        "ones7": np.ones((NOUT, 1), ml_dtypes.bfloat16),
        "ones": np.ones((1, 128), ml_dtypes.bfloat16),
        "onesf": np.ones((1, 128), np.float32),
    }
    in_maps = []
    for c in range(NC):
        m = dict(shared)
        m["srcidx"] = wrap_idx(src_slots[c])
        m["ind"] = np.ascontiguousarray(ind[c])
        m["ind2"] = np.ascontiguousarray(ind2[c])
        m["xT_own"] = np.ascontiguousarray(xT[:, c * NPC:(c + 1) * NPC])
        m["deg"] = deg_new[None, c * NPC:(c + 1) * NPC].astype(bf)
        in_maps.append(m)

    zb = all(np.all(np.asarray(v) == 0) for v in
             (p["b1qkv"], p["b2qkv"], p["b1o"], p["b2o"]))
    return {
        "T_BLK": T_BLK, "NT": NT, "c1": c1, "c2": c2, "c3": c3,
        "in_maps": in_maps, "new_of_old": new_of_old,
        "h1_host": h1_h, "h2_host": h2_h, "zero_bias": zb,
    }


def _build_graph(T_BLK, NT, c1, c2, c3, debug_stop=None,
                 zero_bias=False):
    nc = bacc.Bacc("TRN2")

    xT_d = nc.declare_dram_parameter("xT", [128, NP], BF16, False)
    srcidx_d = nc.declare_dram_parameter("srcidx", [128, NT * 8], I16, False)
    ind_d = nc.declare_dram_parameter("ind", [128, NT * 128], BF16, False)
    ind2_d = nc.declare_dram_parameter("ind2", [128, NT * 128], BF16, False)
    xTown_d = nc.declare_dram_parameter("xT_own", [128, NPC], BF16, False)
    deg_d = nc.declare_dram_parameter("deg", [1, NPC], BF16, False)
    W1_d = nc.declare_dram_parameter("W1", [128, 384], BF16, False)
    W2_d = nc.declare_dram_parameter("W2", [128, 384], BF16, False)
    brow1_d = nc.declare_dram_parameter("brow1", [1, 384], BF16, False)
    brow2_d = nc.declare_dram_parameter("brow2", [1, 384], BF16, False)
    Wo1_d = nc.declare_dram_parameter("Wo1", [128, 128], BF16, False)
    Wo2_d = nc.declare_dram_parameter("Wo2", [128, 128], BF16, False)
    bocol1_d = nc.declare_dram_parameter("bocol1", [1, 128], BF16, False)
    bocol2_d = nc.declare_dram_parameter("bocol2", [1, 128], BF16, False)
    WoutT_d = nc.declare_dram_parameter("WoutT", [128, NOUT], BF16, False)
    outbc_d = nc.declare_dram_parameter("outbc", [1, NOUT], F32, False)
    nones7_d = nc.declare_dram_parameter("nones7", [1, NOUT], F32, False)
    ones7_d = nc.declare_dram_parameter("ones7", [NOUT, 1], BF16, False)
    ones_d = nc.declare_dram_parameter("ones", [1, 128], BF16, False)
    onesf_d = nc.declare_dram_parameter("onesf", [1, 128], F32, False)
    logits_d = nc.declare_dram_parameter("logits", [NOUT, NPC], F32, True)
    dbg_d = nc.declare_dram_parameter("dbg", [128, NPC], BF16, True) \
        if debug_stop else None

    kvtab1 = nc.dram_tensor("kvtab1", [NP, 256], BF16)
    kvtab2 = nc.dram_tensor("kvtab2", [NP, 256], BF16)
    h1b = nc.dram_tensor("h1b", [128, NPC], BF16)
    h1full_d = nc.dram_tensor("h1full", [NC, 128, NPC], BF16,
                              addr_space="Shared")

    with tile.TileContext(nc) as tc, \
            tc.tile_pool(name="const", bufs=1) as cpool, \
            tc.tile_pool(name="big", bufs=1) as bigpool, \
            tc.tile_pool(name="stg", bufs=2) as stg, \
            tc.tile_pool(name="wk", bufs=1) as wk, \
            tc.tile_pool(name="wk2", bufs=2) as wk2, \
            tc.tile_pool(name="gth", bufs=2) as gth, \
            tc.tile_pool(name="wk3", bufs=3) as wk3, \
            tc.tile_pool(name="ps", bufs=2, space="PSUM") as pspool, \
            tc.tile_pool(name="ps1", bufs=1, space="PSUM") as pspool1:

        def const_tile(dram, shape, dt, tag):
            t = cpool.tile(shape, dt, tag=tag)
            nc.sync.dma_start(t[:], dram[:])
            return t

        W1_sb = const_tile(W1_d, [128, 384], BF16, "cW1")
        W2_sb = const_tile(W2_d, [128, 384], BF16, "cW2")
        brow1_sb = const_tile(brow1_d, [1, 384], BF16, "cbr1")
        brow2_sb = const_tile(brow2_d, [1, 384], BF16, "cbr2")
        Wo1_sb = const_tile(Wo1_d, [128, 128], BF16, "cWo1")
        Wo2_sb = const_tile(Wo2_d, [128, 128], BF16, "cWo2")
        bocol1_sb = const_tile(bocol1_d, [1, 128], BF16, "cbo1")
        bocol2_sb = const_tile(bocol2_d, [1, 128], BF16, "cbo2")
        WoutT_sb = const_tile(WoutT_d, [128, NOUT], BF16, "cWt")
        outbc_sb = const_tile(outbc_d, [1, NOUT], F32, "cob")
        nones7_sb = const_tile(nones7_d, [1, NOUT], F32, "cn7")
        ones7_sb = const_tile(ones7_d, [NOUT, 1], BF16, "co7")
        ones_sb = const_tile(ones_d, [1, 128], BF16, "co1")
        onesf_sb = const_tile(onesf_d, [1, 128], F32, "cof")
        deg_sb = const_tile(deg_d, [1, NPC], BF16, "cdeg")

        srcidx_sb = bigpool.tile([128, NT * 8], I16, tag="sidx")
        nc.sync.dma_start(srcidx_sb[:], srcidx_d[:])

        h1own_sb = bigpool.tile([128, NPC], BF16, tag="h1own")
        h2own_sb = None
        if debug_stop in (None, "conv2"):
            h2own_sb = bigpool.tile([128, NPC], BF16, tag="h2own")

        def projections(src_dram_stripe, W_sb, brow_sb, kvtab):
            # one stripe = 20 blocks = 2560 nodes; src_dram_stripe(g) returns
            # a [128, 2560] DRAM AP (feature-major input features)
            for g in range(NC):
                inp = stg.tile([128, BPC * 128], BF16, tag="inp")
                nc.sync.dma_start(inp[:], src_dram_stripe(g))
                kvst = stg.tile([128, BPC, 256], BF16, tag="kvst")
                for j in range(BPC):
                    ps = pspool.tile([128, 256], F32, tag="proj")
                    nc.tensor.matmul(ps[:], inp[:, j * 128:(j + 1) * 128],
                                     W_sb[:, 128:384], start=True,
                                     stop=zero_bias)
                    if not zero_bias:
                        nc.tensor.matmul(ps[:], ones_sb[:],
                                         brow_sb[:, 128:384],
                                         start=False, stop=True)
                    if j % 2 == 0:
                        nc.scalar.copy(kvst[:, j, :], ps[:])
                    else:
                        nc.vector.tensor_copy(kvst[:, j, :], ps[:])
                kv = kvtab[g * NPC:(g + 1) * NPC, :].rearrange(
                    "(b p) c -> p b c", p=128)
                nc.sync.dma_start(kv, kvst[:])

        def q_own_pass(src_fm, W_sb, brow_sb, qown):
            # q projection of this core's own stripe, node-major into SBUF
            for j in range(BPC):
                ps = pspool.tile([128, 128], F32, tag="proj")
                nc.tensor.matmul(ps[:], src_fm[:, j * 128:(j + 1) * 128],
                                 W_sb[:, 0:128], start=True, stop=zero_bias)
                if not zero_bias:
                    nc.tensor.matmul(ps[:], ones_sb[:], brow_sb[:, 0:128],
                                     start=False, stop=True)
                if j % 2 == 0:
                    nc.scalar.copy(qown[:, j, :], ps[:])
                else:
                    nc.vector.tensor_copy(qown[:, j, :], ps[:])

        def block_epilogue(blk, aggps, Wo_sb, bocol_sb, hdst):
            aggsb = wk2.tile([128, 128], BF16, tag="aggsb")
            nc.scalar.copy(aggsb[:], aggps[:])
            hps = pspool.tile([128, 128], F32, tag="proj")
            nc.tensor.matmul(hps[:], Wo_sb[:], aggsb[:], start=True,
                             stop=zero_bias)
            if not zero_bias:
                nc.tensor.matmul(hps[:], bocol_sb[:],
                                 deg_sb[:, blk * 128:(blk + 1) * 128],
                                 start=False, stop=True)
            # ELU(x) = relu(x) + exp(min(x,0)) - 1
            t1 = wk2.tile([128, 128], BF16, tag="t1")
            nc.scalar.activation(t1[:], hps[:], AF.Relu)
            t2 = wk2.tile([128, 128], F32, tag="t2")
            nc.vector.tensor_scalar_min(t2[:], hps[:], 0.0)
            t3 = wk2.tile([128, 128], BF16, tag="t3")
            nc.scalar.activation(t3[:], t2[:], AF.Exp)
            nc.vector.scalar_tensor_tensor(
                hdst[:, blk * 128:(blk + 1) * 128], t3[:], -1.0, t1[:],
                ALU.add, ALU.add)

        def attention(kvtab, qown, Wo_sb, bocol_sb, hdst, cshift):
            GB = 8
            ctx_lp = nc.allow_low_precision(
                reason="bf16 tree-adds validated vs reference (rel<2e-2)")
            ctx_lp.__enter__()
            agg_ps = None
            for ch in range(NT // CH):
                kvg = gth.tile([128, CH, 256], BF16, tag="kvg")
                ind_t = gth.tile([128, CH * 128], BF16, tag="ind")
                ind2_t = gth.tile([128, CH * 128], BF16, tag="ind2")
                i0 = ch * CH * 8
                i1 = (ch + 1) * CH * 8
                nc.gpsimd.dma_gather(kvg[:], kvtab[:], srcidx_sb[:, i0:i1],
                                     CH * 128, CH * 128, 256,
                                     single_packet=False)
                nc.sync.dma_start(
                    ind_t[:], ind_d[:, ch * CH * 128:(ch + 1) * CH * 128])
                nc.sync.dma_start(
                    ind2_t[:], ind2_d[:, ch * CH * 128:(ch + 1) * CH * 128])
                for gb in range(CH // GB):
                    prodC = wk2.tile([128, GB, 1024], BF16, tag="bigprod")
                    qexpC = wk3.tile([128, GB, 128], BF16, tag="qexpC")
                    for j in range(GB):
                        ti = gb * GB + j
                        t = ch * CH + ti
                        blk = min(t // T_BLK, BPC - 1)
                        qps = pspool.tile([128, 128], F32, tag="qexp")
                        nc.tensor.matmul(
                            qps[:], ind2_t[:, ti * 128:(ti + 1) * 128],
                            qown[:, blk, :], start=True, stop=True)
                        nc.scalar.copy(qexpC[:, j, :], qps[:])
                    for j in range(GB):
                        ti = gb * GB + j
                        q2 = qexpC[:, j, :].rearrange(
                            "p (f dh) -> p f dh", f=8)
                        qb = q2.unsqueeze(2).broadcast_to([128, 8, 8, 16])
                        k2 = kvg[:, ti, 0:128].rearrange(
                            "p (g dh) -> p g dh", g=8)
                        kb = k2.unsqueeze(1).broadcast_to([128, 8, 8, 16])
                        nc.vector.tensor_mul(
                            prodC[:, j, :].rearrange(
                                "p (f g dh) -> p f g dh", f=8, g=8),
                            qb, kb)
                    # batched tree-add over d: prod slot layout (f, g, d, h)
                    pv = prodC[:].rearrange(
                        "p b (fg d h) -> p (b fg) d h", fg=64, d=4)
                    t1C = wk.tile([128, GB, 512], BF16, tag="t1C")
                    nc.vector.tensor_add(
                        t1C[:].rearrange("p b (fg d h) -> p (b fg) d h",
                                         fg=64, d=2),
                        pv[:, :, 0:2, :], pv[:, :, 2:4, :])
                    t1v = t1C[:].rearrange(
                        "p b (fg d h) -> p (b fg) d h", fg=64, d=2)
                    scorC = wk.tile([128, GB, 256], BF16, tag="scorC")
                    nc.vector.tensor_add(
                        scorC[:].rearrange("p b (fg h) -> p (b fg) h", fg=64),
                        t1v[:, :, 0, :], t1v[:, :, 1, :])
                    EC = wk3.tile([128, GB, 256], BF16, tag="EC")
                    nc.scalar.activation(
                        EC[:].rearrange("p b c -> p (b c)"),
                        scorC[:].rearrange("p b c -> p (b c)"),
                        AF.Exp, bias=-cshift)
                    # batched z-tree over g: E slot layout (f, g, h)
                    ev = EC[:].rearrange("p b (f g h) -> p (b f) g h",
                                         f=8, g=8)
                    z1C = wk.tile([128, GB, 128], BF16, tag="z1C")
                    nc.vector.tensor_add(
                        z1C[:].rearrange("p b (f g h) -> p (b f) g h",
                                         f=8, g=4),
                        ev[:, :, 0:4, :], ev[:, :, 4:8, :])
                    z1v = z1C[:].rearrange("p b (f g h) -> p (b f) g h",
                                           f=8, g=4)
                    z2C = wk.tile([128, GB, 64], BF16, tag="z2C")
                    nc.vector.tensor_add(
                        z2C[:].rearrange("p b (f g h) -> p (b f) g h",
                                         f=8, g=2),
                        z1v[:, :, 0:2, :], z1v[:, :, 2:4, :])
                    z2v = z2C[:].rearrange("p b (f g h) -> p (b f) g h",
                                           f=8, g=2)
                    ZtC = wk.tile([128, GB, 32], F32, tag="ZtC")
                    nc.vector.tensor_add(
                        ZtC[:].rearrange("p b (f h) -> p (b f) h", f=8),
                        z2v[:, :, 0, :], z2v[:, :, 1, :])
                    rzf = wk.tile([128, GB, 32], F32, tag="rzf")
                    nc.vector.reciprocal_approx_fast(
                        rzf[:].rearrange("p b c -> p (b c)"),
                        ZtC[:].rearrange("p b c -> p (b c)"))
                    rzC = wk.tile([128, GB, 32], BF16, tag="rzC")
                    nc.scalar.copy(
                        rzC[:].rearrange("p b c -> p (b c)"),
                        rzf[:].rearrange("p b c -> p (b c)"))
                    # AV products per tile: pav slot layout (f, d, (g,h))
                    pavC = wk2.tile([128, GB, 1024], BF16, tag="bigprod")
                    for j in range(GB):
                        ti = gb * GB + j
                        eb = EC[:, j, :].rearrange("p (f gh) -> p f gh", f=8) \
                            .unsqueeze(2).broadcast_to([128, 8, 4, 32])
                        vb = kvg[:, ti, 128:256].rearrange(
                            "p (d gh) -> p d gh", d=4) \
                            .unsqueeze(1).broadcast_to([128, 8, 4, 32])
                        nc.vector.tensor_mul(
                            pavC[:, j, :].rearrange(
                                "p (f d gh) -> p f d gh", f=8, d=4),
                            eb, vb)
                    # batched o-tree over g: pav slot (f, d, g, h)
                    ov = pavC[:].rearrange("p b (fd g h) -> p (b fd) g h",
                                           fd=32, g=8)
                    o1C = wk.tile([128, GB, 512], BF16, tag="o1C")
                    nc.vector.tensor_add(
                        o1C[:].rearrange("p b (fd g h) -> p (b fd) g h",
                                         fd=32, g=4),
                        ov[:, :, 0:4, :], ov[:, :, 4:8, :])
                    o1v = o1C[:].rearrange("p b (fd g h) -> p (b fd) g h",
                                           fd=32, g=4)
                    o2C = wk.tile([128, GB, 256], BF16, tag="o2C")
                    nc.vector.tensor_add(
                        o2C[:].rearrange("p b (fd g h) -> p (b fd) g h",
                                         fd=32, g=2),
                        o1v[:, :, 0:2, :], o1v[:, :, 2:4, :])
                    o2v = o2C[:].rearrange("p b (fd g h) -> p (b fd) g h",
                                           fd=32, g=2)
                    OfC = wk.tile([128, GB, 128], BF16, tag="OfC")
                    nc.vector.tensor_add(
                        OfC[:].rearrange("p b (fd h) -> p (b fd) h", fd=32),
                        o2v[:, :, 0, :], o2v[:, :, 1, :])
                    # batched normalize: Ot[b,f,d,h] = Of[b,f,d,h]*rz[b,f,h]
                    OtC = wk2.tile([128, GB, 128], BF16, tag="OtC")
                    rzb = rzC[:].rearrange("p b (f h) -> p (b f) h", f=8) \
                        .unsqueeze(2).broadcast_to([128, GB * 8, 4, 4])
                    nc.vector.tensor_mul(
                        OtC[:].rearrange("p b (f d h) -> p (b f) d h",
                                         f=8, d=4),
                        OfC[:].rearrange("p b (f d h) -> p (b f) d h",
                                         f=8, d=4),
                        rzb)
                    # aggregation matmuls per tile
                    for j in range(GB):
                        ti = gb * GB + j
                        t = ch * CH + ti
                        blk = min(t // T_BLK, BPC - 1)
                        is_first = (t == blk * T_BLK)
                        is_last = (t == (blk + 1) * T_BLK - 1) \
                            if blk < BPC - 1 else (t == NT - 1)
                        if is_first:
                            agg_ps = pspool.tile([128, 128], F32, tag="agg")
                        nc.tensor.matmul(agg_ps[:], OtC[:, j, :],
                                         ind_t[:, ti * 128:(ti + 1) * 128],
                                         start=is_first, stop=is_last)
                        if is_last:
                            block_epilogue(blk, agg_ps, Wo_sb, bocol_sb, hdst)
            ctx_lp.__exit__(None, None, None)

        # ---------------- conv1 ----------------
        xTown_sb = bigpool.tile([128, NPC], BF16, tag="xtown")
        nc.sync.dma_start(xTown_sb[:], xTown_d[:])
        qown1 = bigpool.tile([128, BPC, 128], BF16, tag="qown1")
        q_own_pass(xTown_sb, W1_sb, brow1_sb, qown1)
        projections(lambda g: xT_d[:, g * NPC:(g + 1) * NPC],
                    W1_sb, brow1_sb, kvtab1)
        if debug_stop == "proj1":
            nc.sync.dma_start(
                dbg_d[:].rearrange("p (b c) -> p b c", c=128),
                kvtab1[0:NPC, 0:128].rearrange("(b p) c -> p b c", p=128))
        if debug_stop is None or debug_stop not in ("proj1",):
            attention(kvtab1, qown1, Wo1_sb, bocol1_sb, h1own_sb, c1)
        if debug_stop == "conv1":
            nc.sync.dma_start(dbg_d[:], h1own_sb[:])

        # ---------------- allgather ----------------
        go2 = debug_stop is None or debug_stop in ("gather1", "conv2")
        if go2:
            nc.sync.dma_start(h1b[:], h1own_sb[:])
        if go2:
            nc.gpsimd.collective_compute(
                "AllGather", ALU.bypass,
                replica_groups=[list(range(NC))],
                ins=[h1b[:]], outs=[h1full_d[:]])
        if debug_stop == "gather1":
            nc.sync.dma_start(dbg_d[:], h1full_d[0, :, :])

        # ---------------- conv2 ----------------
        if debug_stop is None or debug_stop == "conv2":
            qown2 = bigpool.tile([128, BPC, 128], BF16, tag="qown1")
            q_own_pass(h1own_sb, W2_sb, brow2_sb, qown2)
            projections(lambda g: h1full_d[g, :, :],
                        W2_sb, brow2_sb, kvtab2)
            attention(kvtab2, qown2, Wo2_sb, bocol2_sb, h2own_sb, c2)
        if debug_stop == "conv2":
            nc.sync.dma_start(dbg_d[:], h2own_sb[:])

        # ---------------- head + log_softmax ----------------
        hd = debug_stop is None
        for blk in range(BPC if hd else 0):
            sl = slice(blk * 128, (blk + 1) * 128)
            lps = pspool1.tile([NOUT, 128], F32, tag="head")
            nc.tensor.matmul(lps[:], WoutT_sb[:], h2own_sb[:, sl],
                             start=True, stop=False)
            nc.tensor.matmul(lps[:], outbc_sb[:], onesf_sb[:],
                             start=False, stop=True)
            e2 = wk2.tile([NOUT, 128], BF16, tag="e2")
            nc.scalar.activation(e2[:], lps[:], AF.Exp)
            zps = pspool1.tile([1, 128], F32, tag="zs")
            nc.tensor.matmul(zps[:], ones7_sb[:], e2[:], start=True, stop=True)
            lnz = wk2.tile([1, 128], F32, tag="lnz")
            nc.scalar.activation(lnz[:], zps[:], AF.Ln)
            nc.tensor.matmul(lps[:], nones7_sb[:], lnz[:],
                             start=False, stop=True, skip_group_check=True)
            outt = wk2.tile([NOUT, 128], F32, tag="outt")
            nc.vector.tensor_copy(outt[:], lps[:])
            nc.sync.dma_start(logits_d[:, sl], outt[:])

    nc.compile()
    return nc


def kernel(**inputs):
    prep = _host_prep(inputs)
    nc = _build_graph(prep["T_BLK"], prep["NT"],
                      prep["c1"], prep["c2"], prep["c3"],
                      zero_bias=prep["zero_bias"])
    res = run_bass_kernel_spmd(nc, prep["in_maps"], core_ids=list(range(NC)))
    logits = np.concatenate([r["logits"] for r in res.results], axis=1)
    out = logits.T[prep["new_of_old"]]
    return np.ascontiguousarray(out.astype(np.float32))


# revision 34
# speedup vs baseline: 1.1902x; 1.1902x over previous
"""AMPNet GNN classifier on 8 Trainium2 NeuronCores (Bass/Tile).

Sharding: edges are sharded by destination-node range (edge/data parallel on
the dst side). Each core owns 20 blocks of 128 nodes (nodes renumbered on the
host so per-block degree sums are balanced), computes all per-edge attention
messages for edges landing in its range, and aggregates them locally — no
cross-core reduction needed. A bf16 AllGather of the per-core node features
runs between the two conv layers. The small MHA / head weights are replicated.

Per conv layer on each core:
  - TensorE projections build the kv table (node-major bf16 rows in HBM,
    k row (g,d,h) / v row (d,g,h) layouts) for all nodes, and the q table for
    own nodes only (kept in SBUF, node-major).
  - gpsimd dma_gather pulls per-edge kv rows by src id (the only random
    gather); per-edge q is expanded from own-range q via TensorE matmuls with
    a host-built transposed one-hot indicator (q-expansion).
  - VectorE computes the per-edge MHA edge-major, 128 edges per partition
    tile: wide bf16 2x-mode products + tree-adds for the d- and g-reductions,
    ScalarE exp (softmax without max-subtraction; exp-shift constants are
    derived from a host forward pass), fast-reciprocal normalization.
  - TensorE aggregates messages into per-block node sums via host-built
    one-hot indicator matmuls accumulated in PSUM (feature-major output),
    then Wo (block-diagonal) + deg*bias + ELU.
Head: logits + log_softmax feature-major via K=1/K=7 matmul broadcast tricks;
the host reassembles and un-permutes the final [N, 7] output.
"""

import numpy as np
import ml_dtypes

import concourse.bacc as bacc
import concourse.bass as bass
import concourse.mybir as mybir
import concourse.tile as tile
from concourse.bass_utils import run_bass_kernel_spmd

N = 20000
E_TOT = 320000
F = 8
D = 16
H = 4
DH = 4
NOUT = 7
FDIM = 128  # F*D

NC = 8
NP = 20480          # padded node count (160 blocks of 128)
NB = 160            # total 128-node blocks
BPC = NB // NC      # blocks per core
NPC = NP // NC      # nodes per core
CH = 16             # edge tiles per gather chunk

BF16 = mybir.dt.bfloat16
F32 = mybir.dt.float32
I16 = mybir.dt.int16

AF = mybir.ActivationFunctionType
ALU = mybir.AluOpType


def _bd_qkv_weights(wqkv, bqkv):
    """Block-diagonal projection weights [128, 384] + bias row [1, 384].

    Output cols: 0:128   q-table row, col f*16 + (h*4+d)   (scaled by 1/sqrt(DH))
                 128:256 k part of kv row, col g*16 + (h*4+d)
                 256:384 v part of kv row, col h*32 + d*8 + g
    Input rows: token-major x row, f*16 + di.
    """
    wq, wk, wv = wqkv[0:16], wqkv[16:32], wqkv[32:48]
    bq, bk, bv = bqkv[0:16], bqkv[16:32], bqkv[32:48]
    sc = 1.0 / np.sqrt(DH)
    W = np.zeros((128, 384), np.float32)
    brow = np.zeros((1, 384), np.float32)
    # q row layout (f, d, h): col f*16 + d*4 + h
    # k row layout (g, d, h): col 128 + g*16 + d*4 + h
    # v row layout (d, g, h): col 256 + d*32 + g*4 + h
    for f in range(F):
        for h in range(H):
            for d in range(DH):
                W[f * 16:(f + 1) * 16, f * 16 + d * 4 + h] = wq[h * 4 + d] * sc
                W[f * 16:(f + 1) * 16, 128 + f * 16 + d * 4 + h] = wk[h * 4 + d]
                W[f * 16:(f + 1) * 16, 256 + d * 32 + f * 4 + h] = wv[h * 4 + d]
                brow[0, f * 16 + d * 4 + h] = bq[h * 4 + d] * sc
                brow[0, 128 + f * 16 + d * 4 + h] = bk[h * 4 + d]
                brow[0, 256 + d * 32 + f * 4 + h] = bv[h * 4 + d]
    return W, brow


def _bd_wo(wo):
    """Wo block-diag [128, 128]: in row f*16 + d*4 + h -> out col f*16+c."""
    W = np.zeros((128, 128), np.float32)
    for f in range(F):
        for h in range(H):
            for d in range(DH):
                W[f * 16 + d * 4 + h, f * 16:(f + 1) * 16] = wo[:, h * 4 + d]
    return W


def _host_forward(x, src, dst, p):
    """Exact reference forward in numpy (chunked). Returns max scores of both
    convs and max |logit| (for exp-shift constants), plus the logits."""
    def conv(hmat, wqkv, bqkv, wo, bo):
        wq, wk, wv = wqkv[0:16], wqkv[16:32], wqkv[32:48]
        bq, bk, bv = bqkv[0:16], bqkv[16:32], bqkv[32:48]
        h3 = hmat.reshape(N, F, D)
        q = (h3 @ wq.T + bq).astype(np.float32)
        k = (h3 @ wk.T + bk).astype(np.float32)
        v = (h3 @ wv.T + bv).astype(np.float32)
        agg = np.zeros((N, FDIM), np.float32)
        smax = -1e30
        order = np.argsort(dst, kind="stable")
        s_s, d_s = src[order], dst[order]
        msg = np.empty((E_TOT, FDIM), np.float32)
        CHK = 65536
        for lo in range(0, E_TOT, CHK):
            hi = min(lo + CHK, E_TOT)
            qd = q[d_s[lo:hi]].reshape(-1, F, H, DH)
            ks = k[s_s[lo:hi]].reshape(-1, F, H, DH)
            vs = v[s_s[lo:hi]].reshape(-1, F, H, DH)
            s = np.einsum("efhd,eghd->ehfg", qd, ks) * (1.0 / np.sqrt(DH))
            smax = max(smax, float(s.max()))
            s -= s.max(axis=-1, keepdims=True)
            es = np.exp(s)
            a = es / es.sum(axis=-1, keepdims=True)
            o = np.einsum("ehfg,eghd->efhd", a, vs).reshape(-1, FDIM)
            msg[lo:hi] = (o.reshape(-1, F, D) @ wo.T + bo).reshape(-1, FDIM)
        # segment sum over sorted dst
        uniq, starts = np.unique(d_s, return_index=True)
        sums = np.add.reduceat(msg, starts, axis=0)
        agg[uniq] = sums
        return agg, smax

    h1, smax1 = conv(x, p["w1qkv"], p["b1qkv"], p["w1o"], p["b1o"])
    h1 = np.where(h1 > 0, h1, np.expm1(np.minimum(h1, 0)))
    h2, smax2 = conv(h1, p["w2qkv"], p["b2qkv"], p["w2o"], p["b2o"])
    h2 = np.where(h2 > 0, h2, np.expm1(np.minimum(h2, 0)))
    logits = h2 @ p["out_w"].T + p["out_b"]
    return smax1, smax2, float(np.abs(logits).max()), logits, h1, h2


def _host_prep(inputs):
    x = np.asarray(inputs["x"], np.float32)
    ei = np.asarray(inputs["edge_index"], np.int64)
    src, dst = ei[0], ei[1]
    p = {
        "w1qkv": np.asarray(inputs["conv1_wqkv"], np.float32),
        "b1qkv": np.asarray(inputs["conv1_bqkv"], np.float32),
        "w1o": np.asarray(inputs["conv1_wo"], np.float32),
        "b1o": np.asarray(inputs["conv1_bo"], np.float32),
        "w2qkv": np.asarray(inputs["conv2_wqkv"], np.float32),
        "b2qkv": np.asarray(inputs["conv2_bqkv"], np.float32),
        "w2o": np.asarray(inputs["conv2_wo"], np.float32),
        "b2o": np.asarray(inputs["conv2_bo"], np.float32),
        "out_w": np.asarray(inputs["out_w"], np.float32),
        "out_b": np.asarray(inputs["out_b"], np.float32),
    }

    smax1, smax2, lmax, _logits_h, h1_h, h2_h = _host_forward(x, src, dst, p)
    c1 = max(0.0, smax1 - 10.0)
    c2 = max(0.0, smax2 - 10.0)
    c3 = max(0.0, lmax - 10.0)

    # ---- node renumbering: degree-balanced blocks (snake over sorted deg) ---
    deg = np.bincount(dst, minlength=N)
    order = np.argsort(-deg, kind="stable")
    rounds = N // NB  # 125
    pos = np.arange(N) % NB
    rnd = np.arange(N) // NB
    blk_of_rank = np.where(rnd % 2 == 0, pos, NB - 1 - pos)
    new_id_of_rank = blk_of_rank * 128 + rnd
    new_of_old = np.empty(N, np.int64)
    new_of_old[order] = new_id_of_rank

    s_new = new_of_old[src]
    d_new = new_of_old[dst]

    blk = d_new >> 7
    order_e = np.argsort(blk, kind="stable")
    s_s = s_new[order_e]
    d_s = d_new[order_e]
    blk_s = blk[order_e]
    cnt = np.bincount(blk_s, minlength=NB)
    T_BLK = int(np.ceil(cnt.max() / 128))
    EPB = T_BLK * 128
    NT0 = BPC * T_BLK
    NT = ((NT0 + CH - 1) // CH) * CH
    ES = NT * 128

    starts = np.zeros(NB, np.int64)
    starts[1:] = np.cumsum(cnt)[:-1]
    within = np.arange(E_TOT) - starts[blk_s]
    core_e = blk_s // BPC
    lb_e = blk_s % BPC
    slot = lb_e * EPB + within

    src_slots = np.zeros((NC, ES), np.int32)
    dst_slots = np.zeros((NC, ES), np.int32)
    # default dst for padding: base node of the slot's block (or last block)
    slot_blk = np.minimum(np.arange(ES) >> 7 >> 0, 0)  # placeholder
    for c in range(NC):
        lb_all = np.minimum(np.arange(ES) // EPB, BPC - 1)
        dst_slots[c] = (c * BPC + lb_all) * 128
    src_slots[core_e, slot] = s_s
    dst_slots[core_e, slot] = d_s

    ind = np.zeros((NC, 128, NT * 128), ml_dtypes.bfloat16)
    pp = slot & 127
    dloc = d_s - blk_s * 128
    tcol = (slot >> 7) * 128 + dloc
    ind[core_e, pp, tcol] = 1.0
    # transposed indicator for q-expansion: [node-in-block, edge-slot]
    ind2 = np.zeros((NC, 128, NT * 128), ml_dtypes.bfloat16)
    ind2[core_e, dloc, slot] = 1.0

    def wrap_idx(a):
        return np.ascontiguousarray(
            np.tile(a.reshape(-1, 16).T.astype(np.int16), (8, 1)))

    deg_new = np.bincount(d_new, minlength=NP).astype(np.float32)

    xp = np.zeros((NP, FDIM), np.float32)
    xp[new_of_old] = x
    xT = np.ascontiguousarray(xp.T).astype(ml_dtypes.bfloat16)

    W1, brow1 = _bd_qkv_weights(p["w1qkv"], p["b1qkv"])
    W2, brow2 = _bd_qkv_weights(p["w2qkv"], p["b2qkv"])
    Wo1 = _bd_wo(p["w1o"])
    Wo2 = _bd_wo(p["w2o"])
    bo_col1 = np.tile(p["b1o"], F)[None, :]  # [1,128] col f*16+c
    bo_col2 = np.tile(p["b2o"], F)[None, :]

    bf = ml_dtypes.bfloat16
    shared = {
        "xT": xT,
        "W1": W1.astype(bf), "W2": W2.astype(bf),
        "brow1": brow1.astype(bf), "brow2": brow2.astype(bf),
        "Wo1": Wo1.astype(bf), "Wo2": Wo2.astype(bf),
        "bocol1": bo_col1.astype(bf), "bocol2": bo_col2.astype(bf),
        "WoutT": np.ascontiguousarray(p["out_w"].T).astype(ml_dtypes.bfloat16),
        "outbc": (p["out_b"] - c3)[None, :].astype(np.float32),
        "nones7": np.full((1, NOUT), -1.0, np.float32),
        "ones7": np.ones((NOUT, 1), ml_dtypes.bfloat16),
        "ones": np.ones((1, 128), ml_dtypes.bfloat16),
        "onesf": np.ones((1, 128), np.float32),
    }
    in_maps = []
    for c in range(NC):
        m = dict(shared)
        m["srcidx"] = wrap_idx(src_slots[c])
        m["ind"] = np.ascontiguousarray(ind[c])
        m["ind2"] = np.ascontiguousarray(ind2[c])
        m["xT_own"] = np.ascontiguousarray(xT[:, c * NPC:(c + 1) * NPC])
        m["deg"] = deg_new[None, c * NPC:(c + 1) * NPC].astype(bf)
        in_maps.append(m)

    zb = all(np.all(np.asarray(v) == 0) for v in
             (p["b1qkv"], p["b2qkv"], p["b1o"], p["b2o"]))
    return {
        "T_BLK": T_BLK, "NT": NT, "c1": c1, "c2": c2, "c3": c3,
        "in_maps": in_maps, "new_of_old": new_of_old,
        "h1_host": h1_h, "h2_host": h2_h, "zero_bias": zb,
    }


def _build_graph(T_BLK, NT, c1, c2, c3, debug_stop=None,
                 zero_bias=False):
    nc = bacc.Bacc("TRN2")

    xT_d = nc.declare_dram_parameter("xT", [128, NP], BF16, False)
    srcidx_d = nc.declare_dram_parameter("srcidx", [128, NT * 8], I16, False)
    ind_d = nc.declare_dram_parameter("ind", [128, NT * 128], BF16, False)
    ind2_d = nc.declare_dram_parameter("ind2", [128, NT * 128], BF16, False)
    xTown_d = nc.declare_dram_parameter("xT_own", [128, NPC], BF16, False)
    deg_d = nc.declare_dram_parameter("deg", [1, NPC], BF16, False)
    W1_d = nc.declare_dram_parameter("W1", [128, 384], BF16, False)
    W2_d = nc.declare_dram_parameter("W2", [128, 384], BF16, False)
    brow1_d = nc.declare_dram_parameter("brow1", [1, 384], BF16, False)
    brow2_d = nc.declare_dram_parameter("brow2", [1, 384], BF16, False)
    Wo1_d = nc.declare_dram_parameter("Wo1", [128, 128], BF16, False)
    Wo2_d = nc.declare_dram_parameter("Wo2", [128, 128], BF16, False)
    bocol1_d = nc.declare_dram_parameter("bocol1", [1, 128], BF16, False)
    bocol2_d = nc.declare_dram_parameter("bocol2", [1, 128], BF16, False)
    WoutT_d = nc.declare_dram_parameter("WoutT", [128, NOUT], BF16, False)
    outbc_d = nc.declare_dram_parameter("outbc", [1, NOUT], F32, False)
    nones7_d = nc.declare_dram_parameter("nones7", [1, NOUT], F32, False)
    ones7_d = nc.declare_dram_parameter("ones7", [NOUT, 1], BF16, False)
    ones_d = nc.declare_dram_parameter("ones", [1, 128], BF16, False)
    onesf_d = nc.declare_dram_parameter("onesf", [1, 128], F32, False)
    logits_d = nc.declare_dram_parameter("logits", [NOUT, NPC], F32, True)
    dbg_d = nc.declare_dram_parameter("dbg", [128, NPC], BF16, True) \
        if debug_stop else None

    kvtab1 = nc.dram_tensor("kvtab1", [NP, 256], BF16)
    kvtab2 = nc.dram_tensor("kvtab2", [NP, 256], BF16)
    h1b = nc.dram_tensor("h1b", [128, NPC], BF16)
    h1full_d = nc.dram_tensor("h1full", [NC, 128, NPC], BF16,
                              addr_space="Shared")

    with tile.TileContext(nc) as tc, \
            tc.tile_pool(name="const", bufs=1) as cpool, \
            tc.tile_pool(name="big", bufs=1) as bigpool, \
            tc.tile_pool(name="stg", bufs=2) as stg, \
            tc.tile_pool(name="wk", bufs=1) as wk, \
            tc.tile_pool(name="wk2", bufs=2) as wk2, \
            tc.tile_pool(name="gth", bufs=2) as gth, \
            tc.tile_pool(name="wk3", bufs=3) as wk3, \
            tc.tile_pool(name="ps", bufs=2, space="PSUM") as pspool, \
            tc.tile_pool(name="ps1", bufs=1, space="PSUM") as pspool1:

        def const_tile(dram, shape, dt, tag):
            t = cpool.tile(shape, dt, tag=tag)
            nc.sync.dma_start(t[:], dram[:])
            return t

        W1_sb = const_tile(W1_d, [128, 384], BF16, "cW1")
        W2_sb = const_tile(W2_d, [128, 384], BF16, "cW2")
        brow1_sb = const_tile(brow1_d, [1, 384], BF16, "cbr1")
        brow2_sb = const_tile(brow2_d, [1, 384], BF16, "cbr2")
        Wo1_sb = const_tile(Wo1_d, [128, 128], BF16, "cWo1")
        Wo2_sb = const_tile(Wo2_d, [128, 128], BF16, "cWo2")
        bocol1_sb = const_tile(bocol1_d, [1, 128], BF16, "cbo1")
        bocol2_sb = const_tile(bocol2_d, [1, 128], BF16, "cbo2")
        WoutT_sb = const_tile(WoutT_d, [128, NOUT], BF16, "cWt")
        outbc_sb = const_tile(outbc_d, [1, NOUT], F32, "cob")
        nones7_sb = const_tile(nones7_d, [1, NOUT], F32, "cn7")
        ones7_sb = const_tile(ones7_d, [NOUT, 1], BF16, "co7")
        ones_sb = const_tile(ones_d, [1, 128], BF16, "co1")
        onesf_sb = const_tile(onesf_d, [1, 128], F32, "cof")
        deg_sb = const_tile(deg_d, [1, NPC], BF16, "cdeg")

        srcidx_sb = bigpool.tile([128, NT * 8], I16, tag="sidx")
        nc.sync.dma_start(srcidx_sb[:], srcidx_d[:])

        h1own_sb = bigpool.tile([128, NPC], BF16, tag="h1own")
        h2own_sb = None
        if debug_stop in (None, "conv2"):
            h2own_sb = bigpool.tile([128, NPC], BF16, tag="h2own")

        def projections(src_dram_stripe, W_sb, brow_sb, kvtab):
            # one stripe = 20 blocks = 2560 nodes; src_dram_stripe(g) returns
            # a [128, 2560] DRAM AP (feature-major input features)
            for g in range(NC):
                inp = stg.tile([128, BPC * 128], BF16, tag="inp")
                nc.sync.dma_start(inp[:], src_dram_stripe(g))
                kvst = stg.tile([128, BPC, 256], BF16, tag="kvst")
                for j in range(BPC):
                    ps = pspool.tile([128, 256], F32, tag="proj")
                    nc.tensor.matmul(ps[:], inp[:, j * 128:(j + 1) * 128],
                                     W_sb[:, 128:384], start=True,
                                     stop=zero_bias)
                    if not zero_bias:
                        nc.tensor.matmul(ps[:], ones_sb[:],
                                         brow_sb[:, 128:384],
                                         start=False, stop=True)
                    if j % 2 == 0:
                        nc.scalar.copy(kvst[:, j, :], ps[:])
                    else:
                        nc.vector.tensor_copy(kvst[:, j, :], ps[:])
                kv = kvtab[g * NPC:(g + 1) * NPC, :].rearrange(
                    "(b p) c -> p b c", p=128)
                nc.sync.dma_start(kv, kvst[:])

        def q_own_pass(src_fm, W_sb, brow_sb, qown):
            # q projection of this core's own stripe, node-major into SBUF
            for j in range(BPC):
                ps = pspool.tile([128, 128], F32, tag="proj")
                nc.tensor.matmul(ps[:], src_fm[:, j * 128:(j + 1) * 128],
                                 W_sb[:, 0:128], start=True, stop=zero_bias)
                if not zero_bias:
                    nc.tensor.matmul(ps[:], ones_sb[:], brow_sb[:, 0:128],
                                     start=False, stop=True)
                if j % 2 == 0:
                    nc.scalar.copy(qown[:, j, :], ps[:])
                else:
                    nc.vector.tensor_copy(qown[:, j, :], ps[:])

        def block_epilogue(blk, aggps, Wo_sb, bocol_sb, hdst):
            aggsb = wk2.tile([128, 128], BF16, tag="aggsb")
            nc.scalar.copy(aggsb[:], aggps[:])
            hps = pspool.tile([128, 128], F32, tag="proj")
            nc.tensor.matmul(hps[:], Wo_sb[:], aggsb[:], start=True,
                             stop=zero_bias)
            if not zero_bias:
                nc.tensor.matmul(hps[:], bocol_sb[:],
                                 deg_sb[:, blk * 128:(blk + 1) * 128],
                                 start=False, stop=True)
            # ELU(x) = relu(x) + exp(min(x,0)) - 1
            t1 = wk2.tile([128, 128], BF16, tag="t1")
            nc.scalar.activation(t1[:], hps[:], AF.Relu)
            t2 = wk2.tile([128, 128], F32, tag="t2")
            nc.vector.tensor_scalar_min(t2[:], hps[:], 0.0)
            t3 = wk2.tile([128, 128], BF16, tag="t3")
            nc.scalar.activation(t3[:], t2[:], AF.Exp)
            nc.vector.scalar_tensor_tensor(
                hdst[:, blk * 128:(blk + 1) * 128], t3[:], -1.0, t1[:],
                ALU.add, ALU.add)

        def attention(kvtab, qown, Wo_sb, bocol_sb, hdst, cshift):
            GB = 8
            ctx_lp = nc.allow_low_precision(
                reason="bf16 tree-adds validated vs reference (rel<2e-2)")
            ctx_lp.__enter__()
            agg_ps = None
            for ch in range(NT // CH):
                kvg = gth.tile([128, CH, 256], BF16, tag="kvg")
                ind_t = gth.tile([128, CH * 128], BF16, tag="ind")
                ind2_t = gth.tile([128, CH * 128], BF16, tag="ind2")
                i0 = ch * CH * 8
                i1 = (ch + 1) * CH * 8
                nc.gpsimd.dma_gather(kvg[:], kvtab[:], srcidx_sb[:, i0:i1],
                                     CH * 128, CH * 128, 256,
                                     single_packet=False)
                nc.sync.dma_start(
                    ind_t[:], ind_d[:, ch * CH * 128:(ch + 1) * CH * 128])
                nc.sync.dma_start(
                    ind2_t[:], ind2_d[:, ch * CH * 128:(ch + 1) * CH * 128])
                for gb in range(CH // GB):
                    prodC = wk2.tile([128, GB, 1024], BF16, tag="bigprod")
                    qexpC = wk3.tile([128, GB, 128], BF16, tag="qexpC")
                    for j in range(GB):
                        ti = gb * GB + j
                        t = ch * CH + ti
                        blk = min(t // T_BLK, BPC - 1)
                        qps = pspool.tile([128, 128], F32, tag="qexp")
                        nc.tensor.matmul(
                            qps[:], ind2_t[:, ti * 128:(ti + 1) * 128],
                            qown[:, blk, :], start=True, stop=True)
                        nc.scalar.copy(qexpC[:, j, :], qps[:])
                    for j in range(GB):
                        ti = gb * GB + j
                        q2 = qexpC[:, j, :].rearrange(
                            "p (f dh) -> p f dh", f=8)
                        qb = q2.unsqueeze(2).broadcast_to([128, 8, 8, 16])
                        k2 = kvg[:, ti, 0:128].rearrange(
                            "p (g dh) -> p g dh", g=8)
                        kb = k2.unsqueeze(1).broadcast_to([128, 8, 8, 16])
                        nc.vector.tensor_mul(
                            prodC[:, j, :].rearrange(
                                "p (f g dh) -> p f g dh", f=8, g=8),
                            qb, kb)
                    # batched tree-add over d: prod slot layout (f, g, d, h)
                    pv = prodC[:].rearrange(
                        "p b (fg d h) -> p (b fg) d h", fg=64, d=4)
                    t1C = wk.tile([128, GB, 512], BF16, tag="t1C")
                    nc.vector.tensor_add(
                        t1C[:].rearrange("p b (fg d h) -> p (b fg) d h",
                                         fg=64, d=2),
                        pv[:, :, 0:2, :], pv[:, :, 2:4, :])
                    t1v = t1C[:].rearrange(
                        "p b (fg d h) -> p (b fg) d h", fg=64, d=2)
                    scorC = wk.tile([128, GB, 256], BF16, tag="scorC")
                    nc.vector.tensor_add(
                        scorC[:].rearrange("p b (fg h) -> p (b fg) h", fg=64),
                        t1v[:, :, 0, :], t1v[:, :, 1, :])
                    EC = wk3.tile([128, GB, 256], BF16, tag="EC")
                    nc.scalar.activation(
                        EC[:].rearrange("p b c -> p (b c)"),
                        scorC[:].rearrange("p b c -> p (b c)"),
                        AF.Exp, bias=-cshift)
                    # batched z-tree over g: E slot layout (f, g, h)
                    ev = EC[:].rearrange("p b (f g h) -> p (b f) g h",
                                         f=8, g=8)
                    z1C = wk.tile([128, GB, 128], BF16, tag="z1C")
                    nc.vector.tensor_add(
                        z1C[:].rearrange("p b (f g h) -> p (b f) g h",
                                         f=8, g=4),
                        ev[:, :, 0:4, :], ev[:, :, 4:8, :])
                    z1v = z1C[:].rearrange("p b (f g h) -> p (b f) g h",
                                           f=8, g=4)
                    z2C = wk.tile([128, GB, 64], BF16, tag="z2C")
                    nc.vector.tensor_add(
                        z2C[:].rearrange("p b (f g h) -> p (b f) g h",
                                         f=8, g=2),
                        z1v[:, :, 0:2, :], z1v[:, :, 2:4, :])
                    z2v = z2C[:].rearrange("p b (f g h) -> p (b f) g h",
                                           f=8, g=2)
                    ZtC = wk.tile([128, GB, 32], F32, tag="ZtC")
                    nc.vector.tensor_add(
                        ZtC[:].rearrange("p b (f h) -> p (b f) h", f=8),
                        z2v[:, :, 0, :], z2v[:, :, 1, :])
                    rzf = wk.tile([128, GB, 32], F32, tag="rzf")
                    nc.vector.reciprocal_approx_fast(
                        rzf[:].rearrange("p b c -> p (b c)"),
                        ZtC[:].rearrange("p b c -> p (b c)"))
                    rzC = wk.tile([128, GB, 32], BF16, tag="rzC")
                    nc.scalar.copy(
                        rzC[:].rearrange("p b c -> p (b c)"),
                        rzf[:].rearrange("p b c -> p (b c)"))
                    # AV products per tile: pav slot layout (f, d, (g,h))
                    pavC = wk2.tile([128, GB, 1024], BF16, tag="bigprod")
                    for j in range(GB):
                        ti = gb * GB + j
                        eb = EC[:, j, :].rearrange("p (f gh) -> p f gh", f=8) \
                            .unsqueeze(2).broadcast_to([128, 8, 4, 32])
                        vb = kvg[:, ti, 128:256].rearrange(
                            "p (d gh) -> p d gh", d=4) \
                            .unsqueeze(1).broadcast_to([128, 8, 4, 32])
                        nc.vector.tensor_mul(
                            pavC[:, j, :].rearrange(
                                "p (f d gh) -> p f d gh", f=8, d=4),
                            eb, vb)
                    # batched o-tree over g: pav slot (f, d, g, h)
                    ov = pavC[:].rearrange("p b (fd g h) -> p (b fd) g h",
                                           fd=32, g=8)
                    o1C = wk.tile([128, GB, 512], BF16, tag="o1C")
                    nc.vector.tensor_add(
                        o1C[:].rearrange("p b (fd g h) -> p (b fd) g h",
                                         fd=32, g=4),
                        ov[:, :, 0:4, :], ov[:, :, 4:8, :])
                    o1v = o1C[:].rearrange("p b (fd g h) -> p (b fd) g h",
                                           fd=32, g=4)
                    o2C = wk.tile([128, GB, 256], BF16, tag="o2C")
                    nc.vector.tensor_add(
                        o2C[:].rearrange("p b (fd g h) -> p (b fd) g h",
                                         fd=32, g=2),
                        o1v[:, :, 0:2, :], o1v[:, :, 2:4, :])
                    o2v = o2C[:].rearrange("p b (fd g h) -> p (b fd) g h",
                                           fd=32, g=2)
                    OfC = wk.tile([128, GB, 128], BF16, tag="OfC")
                    nc.vector.tensor_add(
                        OfC[:].rearrange("p b (fd h) -> p (b fd) h", fd=32),
                        o2v[:, :, 0, :], o2v[:, :, 1, :])
                    # batched normalize: Ot[b,f,d,h] = Of[b,f,d,h]*rz[b,f,h]
                    OtC = wk2.tile([128, GB, 128], BF16, tag="OtC")
                    rzb = rzC[:].rearrange("p b (f h) -> p (b f) h", f=8) \
                        .unsqueeze(2).broadcast_to([128, GB * 8, 4, 4])
                    nc.vector.tensor_mul(
                        OtC[:].rearrange("p b (f d h) -> p (b f) d h",
                                         f=8, d=4),
                        OfC[:].rearrange("p b (f d h) -> p (b f) d h",
                                         f=8, d=4),
                        rzb)
                    # aggregation matmuls per tile
                    for j in range(GB):
                        ti = gb * GB + j
                        t = ch * CH + ti
                        blk = min(t // T_BLK, BPC - 1)
                        is_first = (t == blk * T_BLK)
                        is_last = (t == (blk + 1) * T_BLK - 1) \
                            if blk < BPC - 1 else (t == NT - 1)
                        if is_first:
                            agg_ps = pspool.tile([128, 128], F32, tag="agg")
                        nc.tensor.matmul(agg_ps[:], OtC[:, j, :],
                                         ind_t[:, ti * 128:(ti + 1) * 128],
                                         start=is_first, stop=is_last)
                        if is_last:
                            block_epilogue(blk, agg_ps, Wo_sb, bocol_sb, hdst)
            ctx_lp.__exit__(None, None, None)

        # ---------------- conv1 ----------------
        xTown_sb = bigpool.tile([128, NPC], BF16, tag="xtown")
        nc.sync.dma_start(xTown_sb[:], xTown_d[:])
        qown1 = bigpool.tile([128, BPC, 128], BF16, tag="qown1")
        q_own_pass(xTown_sb, W1_sb, brow1_sb, qown1)
        projections(lambda g: xT_d[:, g * NPC:(g + 1) * NPC],
                    W1_sb, brow1_sb, kvtab1)
        if debug_stop == "proj1":
            nc.sync.dma_start(
                dbg_d[:].rearrange("p (b c) -> p b c", c=128),
                kvtab1[0:NPC, 0:128].rearrange("(b p) c -> p b c", p=128))
        if debug_stop is None or debug_stop not in ("proj1",):
            attention(kvtab1, qown1, Wo1_sb, bocol1_sb, h1own_sb, c1)
        if debug_stop == "conv1":
            nc.sync.dma_start(dbg_d[:], h1own_sb[:])

        # ---------------- allgather ----------------
        go2 = debug_stop is None or debug_stop in ("gather1", "conv2")
        if go2:
            nc.sync.dma_start(h1b[:], h1own_sb[:])
        if go2:
            nc.gpsimd.collective_compute(
                "AllGather", ALU.bypass,
                replica_groups=[list(range(NC))],
                ins=[h1b[:]], outs=[h1full_d[:]])
        if debug_stop == "gather1":
            nc.sync.dma_start(dbg_d[:], h1full_d[0, :, :])

        # ---------------- conv2 ----------------
        if debug_stop is None or debug_stop == "conv2":
            qown2 = bigpool.tile([128, BPC, 128], BF16, tag="qown1")
            q_own_pass(h1own_sb, W2_sb, brow2_sb, qown2)
            projections(lambda g: h1full_d[g, :, :],
                        W2_sb, brow2_sb, kvtab2)
            attention(kvtab2, qown2, Wo2_sb, bocol2_sb, h2own_sb, c2)
        if debug_stop == "conv2":
            nc.sync.dma_start(dbg_d[:], h2own_sb[:])

        # ---------------- head + log_softmax ----------------
        hd = debug_stop is None
        for blk in range(BPC if hd else 0):
            sl = slice(blk * 128, (blk + 1) * 128)
            lps = pspool1.tile([NOUT, 128], F32, tag="head")
            nc.tensor.matmul(lps[:], WoutT_sb[:], h2own_sb[:, sl],
                             start=True, stop=False)
            nc.tensor.matmul(lps[:], outbc_sb[:], onesf_sb[:],
                             start=False, stop=True)
            e2 = wk2.tile([NOUT, 128], BF16, tag="e2")
            nc.scalar.activation(e2[:], lps[:], AF.Exp)
            zps = pspool1.tile([1, 128], F32, tag="zs")
            nc.tensor.matmul(zps[:], ones7_sb[:], e2[:], start=True, stop=True)
            lnz = wk2.tile([1, 128], F32, tag="lnz")
            nc.scalar.activation(lnz[:], zps[:], AF.Ln)
            nc.tensor.matmul(lps[:], nones7_sb[:], lnz[:],
                             start=False, stop=True, skip_group_check=True)
            outt = wk2.tile([NOUT, 128], F32, tag="outt")
            nc.vector.tensor_copy(outt[:], lps[:])
            nc.sync.dma_start(logits_d[:, sl], outt[:])

    nc.compile()
    return nc


def kernel(**inputs):
    prep = _host_prep(inputs)
    nc = _build_graph(prep["T_BLK"], prep["NT"],
                      prep["c1"], prep["c2"], prep["c3"],
                      zero_bias=prep["zero_bias"])
    res = run_bass_kernel_spmd(nc, prep["in_maps"], core_ids=list(range(NC)))
    logits = np.concatenate([r["logits"] for r in res.results], axis=1)
    out = logits.T[prep["new_of_old"]]
    return np.ascontiguousarray(out.astype(np.float32))


# revision 35
# speedup vs baseline: 1.1944x; 1.0036x over previous
"""AMPNet GNN classifier on 8 Trainium2 NeuronCores (Bass/Tile).

Sharding: edges are sharded by destination-node range (edge/data parallel on
the dst side). Each core owns 20 blocks of 128 nodes (nodes renumbered on the
host so per-block degree sums are balanced), computes all per-edge attention
messages for edges landing in its range, and aggregates them locally — no
cross-core reduction needed. A bf16 AllGather of the per-core node features
runs between the two conv layers. The small MHA / head weights are replicated.

Per conv layer on each core:
  - TensorE projections build the kv table (node-major bf16 rows in HBM,
    k row (g,d,h) / v row (d,g,h) layouts) for all nodes, and the q table for
    own nodes only (kept in SBUF, node-major).
  - gpsimd dma_gather pulls per-edge kv rows by src id (the only random
    gather); per-edge q is expanded from own-range q via TensorE matmuls with
    a host-built transposed one-hot indicator (q-expansion).
  - VectorE computes the per-edge MHA edge-major, 128 edges per partition
    tile: wide bf16 2x-mode products + tree-adds for the d- and g-reductions,
    ScalarE exp (softmax without max-subtraction; exp-shift constants are
    derived from a host forward pass), fast-reciprocal normalization.
  - TensorE aggregates messages into per-block node sums via host-built
    one-hot indicator matmuls accumulated in PSUM (feature-major output),
    then Wo (block-diagonal) + deg*bias + ELU.
Head: logits + log_softmax feature-major via K=1/K=7 matmul broadcast tricks;
the host reassembles and un-permutes the final [N, 7] output.
"""

import numpy as np
import ml_dtypes

import concourse.bacc as bacc
import concourse.bass as bass
import concourse.mybir as mybir
import concourse.tile as tile
from concourse.bass_utils import run_bass_kernel_spmd

N = 20000
E_TOT = 320000
F = 8
D = 16
H = 4
DH = 4
NOUT = 7
FDIM = 128  # F*D

NC = 8
NP = 20480          # padded node count (160 blocks of 128)
NB = 160            # total 128-node blocks
BPC = NB // NC      # blocks per core
NPC = NP // NC      # nodes per core
CH = 16             # edge tiles per gather chunk

BF16 = mybir.dt.bfloat16
F32 = mybir.dt.float32
I16 = mybir.dt.int16

AF = mybir.ActivationFunctionType
ALU = mybir.AluOpType


def _bd_qkv_weights(wqkv, bqkv):
    """Block-diagonal projection weights [128, 384] + bias row [1, 384].

    Output cols: 0:128   q-table row, col f*16 + (h*4+d)   (scaled by 1/sqrt(DH))
                 128:256 k part of kv row, col g*16 + (h*4+d)
                 256:384 v part of kv row, col h*32 + d*8 + g
    Input rows: token-major x row, f*16 + di.
    """
    wq, wk, wv = wqkv[0:16], wqkv[16:32], wqkv[32:48]
    bq, bk, bv = bqkv[0:16], bqkv[16:32], bqkv[32:48]
    sc = 1.0 / np.sqrt(DH)
    W = np.zeros((128, 384), np.float32)
    brow = np.zeros((1, 384), np.float32)
    # q row layout (f, d, h): col f*16 + d*4 + h
    # k row layout (g, d, h): col 128 + g*16 + d*4 + h
    # v row layout (d, g, h): col 256 + d*32 + g*4 + h
    for f in range(F):
        for h in range(H):
            for d in range(DH):
                W[f * 16:(f + 1) * 16, f * 16 + d * 4 + h] = wq[h * 4 + d] * sc
                W[f * 16:(f + 1) * 16, 128 + f * 16 + d * 4 + h] = wk[h * 4 + d]
                W[f * 16:(f + 1) * 16, 256 + d * 32 + f * 4 + h] = wv[h * 4 + d]
                brow[0, f * 16 + d * 4 + h] = bq[h * 4 + d] * sc
                brow[0, 128 + f * 16 + d * 4 + h] = bk[h * 4 + d]
                brow[0, 256 + d * 32 + f * 4 + h] = bv[h * 4 + d]
    return W, brow


def _bd_wo(wo):
    """Wo block-diag [128, 128]: in row f*16 + d*4 + h -> out col f*16+c."""
    W = np.zeros((128, 128), np.float32)
    for f in range(F):
        for h in range(H):
            for d in range(DH):
                W[f * 16 + d * 4 + h, f * 16:(f + 1) * 16] = wo[:, h * 4 + d]
    return W


def _host_forward(x, src, dst, p):
    """Exact reference forward in numpy (chunked). Returns max scores of both
    convs and max |logit| (for exp-shift constants), plus the logits."""
    def conv(hmat, wqkv, bqkv, wo, bo):
        wq, wk, wv = wqkv[0:16], wqkv[16:32], wqkv[32:48]
        bq, bk, bv = bqkv[0:16], bqkv[16:32], bqkv[32:48]
        h3 = hmat.reshape(N, F, D)
        q = (h3 @ wq.T + bq).astype(np.float32)
        k = (h3 @ wk.T + bk).astype(np.float32)
        v = (h3 @ wv.T + bv).astype(np.float32)
        agg = np.zeros((N, FDIM), np.float32)
        smax = -1e30
        order = np.argsort(dst, kind="stable")
        s_s, d_s = src[order], dst[order]
        msg = np.empty((E_TOT, FDIM), np.float32)
        CHK = 65536
        for lo in range(0, E_TOT, CHK):
            hi = min(lo + CHK, E_TOT)
            qd = q[d_s[lo:hi]].reshape(-1, F, H, DH)
            ks = k[s_s[lo:hi]].reshape(-1, F, H, DH)
            vs = v[s_s[lo:hi]].reshape(-1, F, H, DH)
            s = np.einsum("efhd,eghd->ehfg", qd, ks) * (1.0 / np.sqrt(DH))
            smax = max(smax, float(s.max()))
            s -= s.max(axis=-1, keepdims=True)
            es = np.exp(s)
            a = es / es.sum(axis=-1, keepdims=True)
            o = np.einsum("ehfg,eghd->efhd", a, vs).reshape(-1, FDIM)
            msg[lo:hi] = (o.reshape(-1, F, D) @ wo.T + bo).reshape(-1, FDIM)
        # segment sum over sorted dst
        uniq, starts = np.unique(d_s, return_index=True)
        sums = np.add.reduceat(msg, starts, axis=0)
        agg[uniq] = sums
        return agg, smax

    h1, smax1 = conv(x, p["w1qkv"], p["b1qkv"], p["w1o"], p["b1o"])
    h1 = np.where(h1 > 0, h1, np.expm1(np.minimum(h1, 0)))
    h2, smax2 = conv(h1, p["w2qkv"], p["b2qkv"], p["w2o"], p["b2o"])
    h2 = np.where(h2 > 0, h2, np.expm1(np.minimum(h2, 0)))
    logits = h2 @ p["out_w"].T + p["out_b"]
    return smax1, smax2, float(np.abs(logits).max()), logits, h1, h2


def _host_prep(inputs):
    x = np.asarray(inputs["x"], np.float32)
    ei = np.asarray(inputs["edge_index"], np.int64)
    src, dst = ei[0], ei[1]
    p = {
        "w1qkv": np.asarray(inputs["conv1_wqkv"], np.float32),
        "b1qkv": np.asarray(inputs["conv1_bqkv"], np.float32),
        "w1o": np.asarray(inputs["conv1_wo"], np.float32),
        "b1o": np.asarray(inputs["conv1_bo"], np.float32),
        "w2qkv": np.asarray(inputs["conv2_wqkv"], np.float32),
        "b2qkv": np.asarray(inputs["conv2_bqkv"], np.float32),
        "w2o": np.asarray(inputs["conv2_wo"], np.float32),
        "b2o": np.asarray(inputs["conv2_bo"], np.float32),
        "out_w": np.asarray(inputs["out_w"], np.float32),
        "out_b": np.asarray(inputs["out_b"], np.float32),
    }

    smax1, smax2, lmax, _logits_h, h1_h, h2_h = _host_forward(x, src, dst, p)
    c1 = max(0.0, smax1 - 10.0)
    c2 = max(0.0, smax2 - 10.0)
    c3 = max(0.0, lmax - 10.0)

    # ---- node renumbering: degree-balanced blocks (snake over sorted deg) ---
    deg = np.bincount(dst, minlength=N)
    order = np.argsort(-deg, kind="stable")
    rounds = N // NB  # 125
    pos = np.arange(N) % NB
    rnd = np.arange(N) // NB
    blk_of_rank = np.where(rnd % 2 == 0, pos, NB - 1 - pos)
    new_id_of_rank = blk_of_rank * 128 + rnd
    new_of_old = np.empty(N, np.int64)
    new_of_old[order] = new_id_of_rank

    s_new = new_of_old[src]
    d_new = new_of_old[dst]

    blk = d_new >> 7
    order_e = np.argsort(blk, kind="stable")
    s_s = s_new[order_e]
    d_s = d_new[order_e]
    blk_s = blk[order_e]
    cnt = np.bincount(blk_s, minlength=NB)
    T_BLK = int(np.ceil(cnt.max() / 128))
    EPB = T_BLK * 128
    NT0 = BPC * T_BLK
    NT = ((NT0 + CH - 1) // CH) * CH
    ES = NT * 128

    starts = np.zeros(NB, np.int64)
    starts[1:] = np.cumsum(cnt)[:-1]
    within = np.arange(E_TOT) - starts[blk_s]
    core_e = blk_s // BPC
    lb_e = blk_s % BPC
    slot = lb_e * EPB + within

    src_slots = np.zeros((NC, ES), np.int32)
    dst_slots = np.zeros((NC, ES), np.int32)
    # default dst for padding: base node of the slot's block (or last block)
    slot_blk = np.minimum(np.arange(ES) >> 7 >> 0, 0)  # placeholder
    for c in range(NC):
        lb_all = np.minimum(np.arange(ES) // EPB, BPC - 1)
        dst_slots[c] = (c * BPC + lb_all) * 128
    src_slots[core_e, slot] = s_s
    dst_slots[core_e, slot] = d_s

    ind = np.zeros((NC, 128, NT * 128), ml_dtypes.bfloat16)
    pp = slot & 127
    dloc = d_s - blk_s * 128
    tcol = (slot >> 7) * 128 + dloc
    ind[core_e, pp, tcol] = 1.0
    # transposed indicator for q-expansion: [node-in-block, edge-slot]
    ind2 = np.zeros((NC, 128, NT * 128), ml_dtypes.bfloat16)
    ind2[core_e, dloc, slot] = 1.0

    def wrap_idx(a):
        return np.ascontiguousarray(
            np.tile(a.reshape(-1, 16).T.astype(np.int16), (8, 1)))

    deg_new = np.bincount(d_new, minlength=NP).astype(np.float32)

    xp = np.zeros((NP, FDIM), np.float32)
    xp[new_of_old] = x
    xT = np.ascontiguousarray(xp.T).astype(ml_dtypes.bfloat16)

    W1, brow1 = _bd_qkv_weights(p["w1qkv"], p["b1qkv"])
    W2, brow2 = _bd_qkv_weights(p["w2qkv"], p["b2qkv"])
    Wo1 = _bd_wo(p["w1o"])
    Wo2 = _bd_wo(p["w2o"])
    bo_col1 = np.tile(p["b1o"], F)[None, :]  # [1,128] col f*16+c
    bo_col2 = np.tile(p["b2o"], F)[None, :]

    bf = ml_dtypes.bfloat16
    shared = {
        "xT": xT,
        "W1": W1.astype(bf), "W2": W2.astype(bf),
        "brow1": brow1.astype(bf), "brow2": brow2.astype(bf),
        "Wo1": Wo1.astype(bf), "Wo2": Wo2.astype(bf),
        "bocol1": bo_col1.astype(bf), "bocol2": bo_col2.astype(bf),
        "WoutT": np.ascontiguousarray(p["out_w"].T).astype(ml_dtypes.bfloat16),
        "outbc": (p["out_b"] - c3)[None, :].astype(np.float32),
        "nones7": np.full((1, NOUT), -1.0, np.float32),
        "ones7": np.ones((NOUT, 1), ml_dtypes.bfloat16),
        "ones": np.ones((1, 128), ml_dtypes.bfloat16),
        "onesf": np.ones((1, 128), np.float32),
    }
    in_maps = []
    for c in range(NC):
        m = dict(shared)
        m["srcidx"] = wrap_idx(src_slots[c])
        m["ind"] = np.ascontiguousarray(ind[c])
        m["ind2"] = np.ascontiguousarray(ind2[c])
        m["xT_own"] = np.ascontiguousarray(xT[:, c * NPC:(c + 1) * NPC])
        m["deg"] = deg_new[None, c * NPC:(c + 1) * NPC].astype(bf)
        in_maps.append(m)

    zb = all(np.all(np.asarray(v) == 0) for v in
             (p["b1qkv"], p["b2qkv"], p["b1o"], p["b2o"]))
    return {
        "T_BLK": T_BLK, "NT": NT, "c1": c1, "c2": c2, "c3": c3,
        "in_maps": in_maps, "new_of_old": new_of_old,
        "h1_host": h1_h, "h2_host": h2_h, "zero_bias": zb,
    }


def _build_graph(T_BLK, NT, c1, c2, c3, debug_stop=None,
                 zero_bias=False):
    nc = bacc.Bacc("TRN2")

    xT_d = nc.declare_dram_parameter("xT", [128, NP], BF16, False)
    srcidx_d = nc.declare_dram_parameter("srcidx", [128, NT * 8], I16, False)
    ind_d = nc.declare_dram_parameter("ind", [128, NT * 128], BF16, False)
    ind2_d = nc.declare_dram_parameter("ind2", [128, NT * 128], BF16, False)
    xTown_d = nc.declare_dram_parameter("xT_own", [128, NPC], BF16, False)
    deg_d = nc.declare_dram_parameter("deg", [1, NPC], BF16, False)
    W1_d = nc.declare_dram_parameter("W1", [128, 384], BF16, False)
    W2_d = nc.declare_dram_parameter("W2", [128, 384], BF16, False)
    brow1_d = nc.declare_dram_parameter("brow1", [1, 384], BF16, False)
    brow2_d = nc.declare_dram_parameter("brow2", [1, 384], BF16, False)
    Wo1_d = nc.declare_dram_parameter("Wo1", [128, 128], BF16, False)
    Wo2_d = nc.declare_dram_parameter("Wo2", [128, 128], BF16, False)
    bocol1_d = nc.declare_dram_parameter("bocol1", [1, 128], BF16, False)
    bocol2_d = nc.declare_dram_parameter("bocol2", [1, 128], BF16, False)
    WoutT_d = nc.declare_dram_parameter("WoutT", [128, NOUT], BF16, False)
    outbc_d = nc.declare_dram_parameter("outbc", [1, NOUT], F32, False)
    nones7_d = nc.declare_dram_parameter("nones7", [1, NOUT], F32, False)
    ones7_d = nc.declare_dram_parameter("ones7", [NOUT, 1], BF16, False)
    ones_d = nc.declare_dram_parameter("ones", [1, 128], BF16, False)
    onesf_d = nc.declare_dram_parameter("onesf", [1, 128], F32, False)
    logits_d = nc.declare_dram_parameter("logits", [NOUT, NPC], F32, True)
    dbg_d = nc.declare_dram_parameter("dbg", [128, NPC], BF16, True) \
        if debug_stop else None

    kvtab1 = nc.dram_tensor("kvtab1", [NP, 256], BF16)
    kvtab2 = nc.dram_tensor("kvtab2", [NP, 256], BF16)
    h1b = nc.dram_tensor("h1b", [128, NPC], BF16)
    h1full_d = nc.dram_tensor("h1full", [NC, 128, NPC], BF16,
                              addr_space="Shared")

    with tile.TileContext(nc) as tc, \
            tc.tile_pool(name="const", bufs=1) as cpool, \
            tc.tile_pool(name="big", bufs=1) as bigpool, \
            tc.tile_pool(name="stg", bufs=2) as stg, \
            tc.tile_pool(name="wk", bufs=1) as wk, \
            tc.tile_pool(name="wk2", bufs=2) as wk2, \
            tc.tile_pool(name="gth", bufs=2) as gth, \
            tc.tile_pool(name="wk3", bufs=3) as wk3, \
            tc.tile_pool(name="ps", bufs=2, space="PSUM") as pspool, \
            tc.tile_pool(name="ps1", bufs=1, space="PSUM") as pspool1:

        def const_tile(dram, shape, dt, tag):
            t = cpool.tile(shape, dt, tag=tag)
            nc.sync.dma_start(t[:], dram[:])
            return t

        W1_sb = const_tile(W1_d, [128, 384], BF16, "cW1")
        W2_sb = const_tile(W2_d, [128, 384], BF16, "cW2")
        brow1_sb = const_tile(brow1_d, [1, 384], BF16, "cbr1")
        brow2_sb = const_tile(brow2_d, [1, 384], BF16, "cbr2")
        Wo1_sb = const_tile(Wo1_d, [128, 128], BF16, "cWo1")
        Wo2_sb = const_tile(Wo2_d, [128, 128], BF16, "cWo2")
        bocol1_sb = const_tile(bocol1_d, [1, 128], BF16, "cbo1")
        bocol2_sb = const_tile(bocol2_d, [1, 128], BF16, "cbo2")
        WoutT_sb = const_tile(WoutT_d, [128, NOUT], BF16, "cWt")
        outbc_sb = const_tile(outbc_d, [1, NOUT], F32, "cob")
        nones7_sb = const_tile(nones7_d, [1, NOUT], F32, "cn7")
        ones7_sb = const_tile(ones7_d, [NOUT, 1], BF16, "co7")
        ones_sb = const_tile(ones_d, [1, 128], BF16, "co1")
        onesf_sb = const_tile(onesf_d, [1, 128], F32, "cof")
        deg_sb = const_tile(deg_d, [1, NPC], BF16, "cdeg")

        srcidx_sb = bigpool.tile([128, NT * 8], I16, tag="sidx")
        nc.sync.dma_start(srcidx_sb[:], srcidx_d[:])

        h1own_sb = bigpool.tile([128, NPC], BF16, tag="h1own")
        h2own_sb = None
        if debug_stop in (None, "conv2"):
            h2own_sb = bigpool.tile([128, NPC], BF16, tag="h2own")

        def projections(src_dram_stripe, W_sb, brow_sb, kvtab):
            # one stripe = 20 blocks = 2560 nodes; src_dram_stripe(g) returns
            # a [128, 2560] DRAM AP (feature-major input features)
            for g in range(NC):
                inp = stg.tile([128, BPC * 128], BF16, tag="inp")
                nc.sync.dma_start(inp[:], src_dram_stripe(g))
                kvst = stg.tile([128, BPC, 256], BF16, tag="kvst")
                for j in range(BPC):
                    ps = pspool.tile([128, 256], F32, tag="proj")
                    nc.tensor.matmul(ps[:], inp[:, j * 128:(j + 1) * 128],
                                     W_sb[:, 128:384], start=True,
                                     stop=zero_bias)
                    if not zero_bias:
                        nc.tensor.matmul(ps[:], ones_sb[:],
                                         brow_sb[:, 128:384],
                                         start=False, stop=True)
                    if j % 2 == 0:
                        nc.scalar.copy(kvst[:, j, :], ps[:])
                    else:
                        nc.vector.tensor_copy(kvst[:, j, :], ps[:])
                kv = kvtab[g * NPC:(g + 1) * NPC, :].rearrange(
                    "(b p) c -> p b c", p=128)
                nc.sync.dma_start(kv, kvst[:])

        def q_own_pass(src_fm, W_sb, brow_sb, qown):
            # q projection of this core's own stripe, node-major into SBUF
            for j in range(BPC):
                ps = pspool.tile([128, 128], F32, tag="proj")
                nc.tensor.matmul(ps[:], src_fm[:, j * 128:(j + 1) * 128],
                                 W_sb[:, 0:128], start=True, stop=zero_bias)
                if not zero_bias:
                    nc.tensor.matmul(ps[:], ones_sb[:], brow_sb[:, 0:128],
                                     start=False, stop=True)
                if j % 2 == 0:
                    nc.scalar.copy(qown[:, j, :], ps[:])
                else:
                    nc.vector.tensor_copy(qown[:, j, :], ps[:])

        def block_epilogue(blk, aggps, Wo_sb, bocol_sb, hdst):
            aggsb = wk2.tile([128, 128], BF16, tag="aggsb")
            nc.scalar.copy(aggsb[:], aggps[:])
            hps = pspool.tile([128, 128], F32, tag="proj")
            nc.tensor.matmul(hps[:], Wo_sb[:], aggsb[:], start=True,
                             stop=zero_bias)
            if not zero_bias:
                nc.tensor.matmul(hps[:], bocol_sb[:],
                                 deg_sb[:, blk * 128:(blk + 1) * 128],
                                 start=False, stop=True)
            # ELU(x) = relu(x) + exp(min(x,0)) - 1
            t1 = wk2.tile([128, 128], BF16, tag="t1")
            nc.scalar.activation(t1[:], hps[:], AF.Relu)
            t2 = wk2.tile([128, 128], F32, tag="t2")
            nc.vector.tensor_scalar_min(t2[:], hps[:], 0.0)
            t3 = wk2.tile([128, 128], BF16, tag="t3")
            nc.scalar.activation(t3[:], t2[:], AF.Exp)
            nc.vector.scalar_tensor_tensor(
                hdst[:, blk * 128:(blk + 1) * 128], t3[:], -1.0, t1[:],
                ALU.add, ALU.add)

        def attention(kvtab, qown, Wo_sb, bocol_sb, hdst, cshift):
            GB = 8
            ctx_lp = nc.allow_low_precision(
                reason="bf16 tree-adds validated vs reference (rel<2e-2)")
            ctx_lp.__enter__()
            agg_ps = None
            for ch in range(NT // CH):
                kvg = gth.tile([128, CH, 256], BF16, tag="kvg")
                ind_t = gth.tile([128, CH * 128], BF16, tag="ind")
                ind2_t = gth.tile([128, CH * 128], BF16, tag="ind2")
                i0 = ch * CH * 8
                i1 = (ch + 1) * CH * 8
                nc.gpsimd.dma_gather(kvg[:], kvtab[:], srcidx_sb[:, i0:i1],
                                     CH * 128, CH * 128, 256,
                                     single_packet=False)
                nc.sync.dma_start(
                    ind_t[:], ind_d[:, ch * CH * 128:(ch + 1) * CH * 128])
                nc.sync.dma_start(
                    ind2_t[:], ind2_d[:, ch * CH * 128:(ch + 1) * CH * 128])
                for gb in range(CH // GB):
                    prodC = wk2.tile([128, GB, 1024], BF16, tag="bigprod")
                    qexpC = wk3.tile([128, GB, 128], BF16, tag="qexpC")
                    for j2 in range(GB // 2):
                        qps = pspool.tile([128, 2, 128], F32, tag="qexp")
                        for h2 in range(2):
                            j = j2 * 2 + h2
                            ti = gb * GB + j
                            t = ch * CH + ti
                            blk = min(t // T_BLK, BPC - 1)
                            nc.tensor.matmul(
                                qps[:, h2, :],
                                ind2_t[:, ti * 128:(ti + 1) * 128],
                                qown[:, blk, :], start=True, stop=True)
                        nc.scalar.copy(qexpC[:, j2 * 2:j2 * 2 + 2, :], qps[:])
                    for j in range(GB):
                        ti = gb * GB + j
                        q2 = qexpC[:, j, :].rearrange(
                            "p (f dh) -> p f dh", f=8)
                        qb = q2.unsqueeze(2).broadcast_to([128, 8, 8, 16])
                        k2 = kvg[:, ti, 0:128].rearrange(
                            "p (g dh) -> p g dh", g=8)
                        kb = k2.unsqueeze(1).broadcast_to([128, 8, 8, 16])
                        nc.vector.tensor_mul(
                            prodC[:, j, :].rearrange(
                                "p (f g dh) -> p f g dh", f=8, g=8),
                            qb, kb)
                    # batched tree-add over d: prod slot layout (f, g, d, h)
                    pv = prodC[:].rearrange(
                        "p b (fg d h) -> p (b fg) d h", fg=64, d=4)
                    t1C = wk.tile([128, GB, 512], BF16, tag="t1C")
                    nc.vector.tensor_add(
                        t1C[:].rearrange("p b (fg d h) -> p (b fg) d h",
                                         fg=64, d=2),
                        pv[:, :, 0:2, :], pv[:, :, 2:4, :])
                    t1v = t1C[:].rearrange(
                        "p b (fg d h) -> p (b fg) d h", fg=64, d=2)
                    scorC = wk.tile([128, GB, 256], BF16, tag="scorC")
                    nc.vector.tensor_add(
                        scorC[:].rearrange("p b (fg h) -> p (b fg) h", fg=64),
                        t1v[:, :, 0, :], t1v[:, :, 1, :])
                    EC = wk3.tile([128, GB, 256], BF16, tag="EC")
                    nc.scalar.activation(
                        EC[:].rearrange("p b c -> p (b c)"),
                        scorC[:].rearrange("p b c -> p (b c)"),
                        AF.Exp, bias=-cshift)
                    # batched z-tree over g: E slot layout (f, g, h)
                    ev = EC[:].rearrange("p b (f g h) -> p (b f) g h",
                                         f=8, g=8)
                    z1C = wk.tile([128, GB, 128], BF16, tag="z1C")
                    nc.vector.tensor_add(
                        z1C[:].rearrange("p b (f g h) -> p (b f) g h",
                                         f=8, g=4),
                        ev[:, :, 0:4, :], ev[:, :, 4:8, :])
                    z1v = z1C[:].rearrange("p b (f g h) -> p (b f) g h",
                                           f=8, g=4)
                    z2C = wk.tile([128, GB, 64], BF16, tag="z2C")
                    nc.vector.tensor_add(
                        z2C[:].rearrange("p b (f g h) -> p (b f) g h",
                                         f=8, g=2),
                        z1v[:, :, 0:2, :], z1v[:, :, 2:4, :])
                    z2v = z2C[:].rearrange("p b (f g h) -> p (b f) g h",
                                           f=8, g=2)
                    ZtC = wk.tile([128, GB, 32], F32, tag="ZtC")
                    nc.vector.tensor_add(
                        ZtC[:].rearrange("p b (f h) -> p (b f) h", f=8),
                        z2v[:, :, 0, :], z2v[:, :, 1, :])
                    rzf = wk.tile([128, GB, 32], F32, tag="rzf")
                    nc.vector.reciprocal_approx_fast(
                        rzf[:].rearrange("p b c -> p (b c)"),
                        ZtC[:].rearrange("p b c -> p (b c)"))
                    rzC = wk.tile([128, GB, 32], BF16, tag="rzC")
                    nc.scalar.copy(
                        rzC[:].rearrange("p b c -> p (b c)"),
                        rzf[:].rearrange("p b c -> p (b c)"))
                    # AV products per tile: pav slot layout (f, d, (g,h))
                    pavC = wk2.tile([128, GB, 1024], BF16, tag="bigprod")
                    for j in range(GB):
                        ti = gb * GB + j
                        eb = EC[:, j, :].rearrange("p (f gh) -> p f gh", f=8) \
                            .unsqueeze(2).broadcast_to([128, 8, 4, 32])
                        vb = kvg[:, ti, 128:256].rearrange(
                            "p (d gh) -> p d gh", d=4) \
                            .unsqueeze(1).broadcast_to([128, 8, 4, 32])
                        nc.vector.tensor_mul(
                            pavC[:, j, :].rearrange(
                                "p (f d gh) -> p f d gh", f=8, d=4),
                            eb, vb)
                    # batched o-tree over g: pav slot (f, d, g, h)
                    ov = pavC[:].rearrange("p b (fd g h) -> p (b fd) g h",
                                           fd=32, g=8)
                    o1C = wk.tile([128, GB, 512], BF16, tag="o1C")
                    nc.vector.tensor_add(
                        o1C[:].rearrange("p b (fd g h) -> p (b fd) g h",
                                         fd=32, g=4),
                        ov[:, :, 0:4, :], ov[:, :, 4:8, :])
                    o1v = o1C[:].rearrange("p b (fd g h) -> p (b fd) g h",
                                           fd=32, g=4)
                    o2C = wk.tile([128, GB, 256], BF16, tag="o2C")
                    nc.vector.tensor_add(
                        o2C[:].rearrange("p b (fd g h) -> p (b fd) g h",
                                         fd=32, g=2),
                        o1v[:, :, 0:2, :], o1v[:, :, 2:4, :])
                    o2v = o2C[:].rearrange("p b (fd g h) -> p (b fd) g h",
                                           fd=32, g=2)
                    OfC = wk.tile([128, GB, 128], BF16, tag="OfC")
                    nc.vector.tensor_add(
                        OfC[:].rearrange("p b (fd h) -> p (b fd) h", fd=32),
                        o2v[:, :, 0, :], o2v[:, :, 1, :])
                    # batched normalize: Ot[b,f,d,h] = Of[b,f,d,h]*rz[b,f,h]
                    OtC = wk2.tile([128, GB, 128], BF16, tag="OtC")
                    rzb = rzC[:].rearrange("p b (f h) -> p (b f) h", f=8) \
                        .unsqueeze(2).broadcast_to([128, GB * 8, 4, 4])
                    nc.vector.tensor_mul(
                        OtC[:].rearrange("p b (f d h) -> p (b f) d h",
                                         f=8, d=4),
                        OfC[:].rearrange("p b (f d h) -> p (b f) d h",
                                         f=8, d=4),
                        rzb)
                    # aggregation matmuls per tile
                    for j in range(GB):
                        ti = gb * GB + j
                        t = ch * CH + ti
                        blk = min(t // T_BLK, BPC - 1)
                        is_first = (t == blk * T_BLK)
                        is_last = (t == (blk + 1) * T_BLK - 1) \
                            if blk < BPC - 1 else (t == NT - 1)
                        if is_first:
                            agg_ps = pspool.tile([128, 128], F32, tag="agg")
                        nc.tensor.matmul(agg_ps[:], OtC[:, j, :],
                                         ind_t[:, ti * 128:(ti + 1) * 128],
                                         start=is_first, stop=is_last)
                        if is_last:
                            block_epilogue(blk, agg_ps, Wo_sb, bocol_sb, hdst)
            ctx_lp.__exit__(None, None, None)

        # ---------------- conv1 ----------------
        xTown_sb = bigpool.tile([128, NPC], BF16, tag="xtown")
        nc.sync.dma_start(xTown_sb[:], xTown_d[:])
        qown1 = bigpool.tile([128, BPC, 128], BF16, tag="qown1")
        q_own_pass(xTown_sb, W1_sb, brow1_sb, qown1)
        projections(lambda g: xT_d[:, g * NPC:(g + 1) * NPC],
                    W1_sb, brow1_sb, kvtab1)
        if debug_stop == "proj1":
            nc.sync.dma_start(
                dbg_d[:].rearrange("p (b c) -> p b c", c=128),
                kvtab1[0:NPC, 0:128].rearrange("(b p) c -> p b c", p=128))
        if debug_stop is None or debug_stop not in ("proj1",):
            attention(kvtab1, qown1, Wo1_sb, bocol1_sb, h1own_sb, c1)
        if debug_stop == "conv1":
            nc.sync.dma_start(dbg_d[:], h1own_sb[:])

        # ---------------- allgather ----------------
        go2 = debug_stop is None or debug_stop in ("gather1", "conv2")
        if go2:
            nc.sync.dma_start(h1b[:], h1own_sb[:])
        if go2:
            nc.gpsimd.collective_compute(
                "AllGather", ALU.bypass,
                replica_groups=[list(range(NC))],
                ins=[h1b[:]], outs=[h1full_d[:]])
        if debug_stop == "gather1":
            nc.sync.dma_start(dbg_d[:], h1full_d[0, :, :])

        # ---------------- conv2 ----------------
        if debug_stop is None or debug_stop == "conv2":
            qown2 = bigpool.tile([128, BPC, 128], BF16, tag="qown1")
            q_own_pass(h1own_sb, W2_sb, brow2_sb, qown2)
            projections(lambda g: h1full_d[g, :, :],
                        W2_sb, brow2_sb, kvtab2)
            attention(kvtab2, qown2, Wo2_sb, bocol2_sb, h2own_sb, c2)
        if debug_stop == "conv2":
            nc.sync.dma_start(dbg_d[:], h2own_sb[:])

        # ---------------- head + log_softmax ----------------
        hd = debug_stop is None
        for blk in range(BPC if hd else 0):
            sl = slice(blk * 128, (blk + 1) * 128)
            lps = pspool1.tile([NOUT, 128], F32, tag="head")
            nc.tensor.matmul(lps[:], WoutT_sb[:], h2own_sb[:, sl],
                             start=True, stop=False)
            nc.tensor.matmul(lps[:], outbc_sb[:], onesf_sb[:],
                             start=False, stop=True)
            e2 = wk2.tile([NOUT, 128], BF16, tag="e2")
            nc.scalar.activation(e2[:], lps[:], AF.Exp)
            zps = pspool1.tile([1, 128], F32, tag="zs")
            nc.tensor.matmul(zps[:], ones7_sb[:], e2[:], start=True, stop=True)
            lnz = wk2.tile([1, 128], F32, tag="lnz")
            nc.scalar.activation(lnz[:], zps[:], AF.Ln)
            nc.tensor.matmul(lps[:], nones7_sb[:], lnz[:],
                             start=False, stop=True, skip_group_check=True)
            outt = wk2.tile([NOUT, 128], F32, tag="outt")
            nc.vector.tensor_copy(outt[:], lps[:])
            nc.sync.dma_start(logits_d[:, sl], outt[:])

    nc.compile()
    return nc


def kernel(**inputs):
    prep = _host_prep(inputs)
    nc = _build_graph(prep["T_BLK"], prep["NT"],
                      prep["c1"], prep["c2"], prep["c3"],
                      zero_bias=prep["zero_bias"])
    res = run_bass_kernel_spmd(nc, prep["in_maps"], core_ids=list(range(NC)))
    logits = np.concatenate([r["logits"] for r in res.results], axis=1)
    out = logits.T[prep["new_of_old"]]
    return np.ascontiguousarray(out.astype(np.float32))
